# revision 1
# baseline (speedup 1.0000x reference)
"""Trainium2 Bass kernel for nn_DUDCLoss_1382979469646.

Data-parallel over the batch dim: 8 cores x 512 rows each. Instead of
materializing the [B, K, C] masked-softmax tensors, the loss is factorized so
each row needs only a handful of C-length passes:

With A=exp(x), E=sum(A), a_k=A[pos_k], En=E-sum_k(a_k), D_j=En+a_j, t_j=eps*D_j:
  xent12_j = log(D2_j) - (G12(t2_j) - S12_j + a1_j*log(a2_j+t2_j)) / D1_j
where G12(t) = sum_c A1_c*log(A2_c+t). The t_j spread around their per-row mean
tbar is O(eps*a_j) and enters only through log(A+t), so G12(t_j) ~= G12(tbar)
to ~1e-8 relative — one C-pass per row-pair direction instead of K.

The multi-label part uses log(sigmoid(x)+eps) ~= u = x - log(1+exp(x)) and
sigmoid(x) = exp(u), so every transcendental stays in the one ACT table set
that holds both Exp and Ln (a patched table-selection policy guarantees a
single ~1.3us table load). The u subtraction runs on the otherwise-idle
gpsimd engine; weighted sums are fused product+accumulate DVE ops
(scalar_tensor_tensor). Products run in bf16 (~2e-5 total rel err vs the
fp64 reference), accumulations in fp32.

Each core writes [128, 12] partial sums; the host does the final tiny
reduction and the para blend.
"""

import numpy as np

NCORES = 8
B, C, K = 4096, 1024, 8
RPC = B // NCORES          # rows per core
P = 128                    # partitions
T = RPC // P               # row-tiles per core
TK = T * K
EPS = 1e-5

_cache = {}


def _patch_act_tables(mybir, bacc):
    """Make the ACT-table-load inserter resolve both Exp and Ln to the one
    set that holds both (natural_log_exp_and_others). The default policy
    picks a singleton set per function, inserting a ~1.3us table load at
    every Exp<->Ln transition in the scheduled stream (13 loads here)."""
    if getattr(bacc, "_dudc_act_patch", False):
        return
    orig = bacc.get_activation_tables
    both = {mybir.ActivationFunctionType.Exp, mybir.ActivationFunctionType.Ln}

    def patched(arch):
        tabs = orig(arch)
        if any(both <= funcs for funcs in tabs.values()):
            for name, funcs in tabs.items():
                if not both <= funcs:
                    funcs.difference_update(both)
        return tabs

    bacc.get_activation_tables = patched
    bacc._dudc_act_patch = True


def _build():
    import concourse.bass as bass
    import concourse.tile as tile
    from concourse import bacc, mybir

    _patch_act_tables(mybir, bacc)

    fp32 = mybir.dt.float32
    bf16 = mybir.dt.bfloat16
    AF = mybir.ActivationFunctionType
    ALU = mybir.AluOpType
    AX = mybir.AxisListType

    nc = bacc.Bacc(
        "TRN2",
        target_bir_lowering=False,
        debug=False,
        num_devices=NCORES,
    )

    x1d = nc.dram_tensor("x1", [RPC, C], fp32, kind="ExternalInput").ap()
    x2d = nc.dram_tensor("x2", [RPC, C], fp32, kind="ExternalInput").ap()
    g1d = nc.dram_tensor("g1", [P, TK], fp32, kind="ExternalInput").ap()
    g2d = nc.dram_tensor("g2", [P, TK], fp32, kind="ExternalInput").ap()
    outd = nc.dram_tensor("out", [P, 3 * T], fp32, kind="ExternalOutput").ap()

    with tile.TileContext(nc) as tc:
        with (
            tc.tile_pool(name="x", bufs=T) as xp,
            tc.tile_pool(name="A", bufs=T) as ap_,
            tc.tile_pool(name="llp", bufs=2) as llpp,
            tc.tile_pool(name="u", bufs=T) as up,
            tc.tile_pool(name="ll", bufs=2) as llp,
            tc.tile_pool(name="sg", bufs=2) as sgp,
            tc.tile_pool(name="scratch", bufs=4) as scp,
            tc.tile_pool(name="small", bufs=1) as sm,
        ):
            # ---- persistent small tiles ----
            gt = sm.tile([P, 2 * TK], fp32)        # g1 | g2
            aa = sm.tile([P, 2 * TK], fp32)        # exp(g1) | exp(g2)
            E1t = sm.tile([P, T], fp32)
            E2t = sm.tile([P, T], fp32)
            P1t = sm.tile([P, T], fp32)
            P2t = sm.tile([P, T], fp32)
            P1s = sm.tile([P, T], fp32)
            P2s = sm.tile([P, T], fp32)
            E1n = sm.tile([P, T], fp32)
            E2n = sm.tile([P, T], fp32)
            tb1 = sm.tile([P, T], fp32)
            tb2 = sm.tile([P, T], fp32)
            SM = sm.tile([P, 4 * TK], fp32)        # a1+tb1 | a2+tb2 | D1 | D2
            LG = sm.tile([P, 4 * TK], fp32)        # ln of SM
            REC = sm.tile([P, 2 * TK], fp32)       # 1/D1 | 1/D2
            Lt = sm.tile([P, 2 * T], fp32)         # L12 | L21 accums
            u12 = sm.tile([P, TK], fp32)
            u21 = sm.tile([P, TK], fp32)
            w12 = sm.tile([P, TK], fp32)
            w21 = sm.tile([P, TK], fp32)
            S12 = sm.tile([P, T], fp32)
            S21 = sm.tile([P, T], fp32)
            W12 = sm.tile([P, T], fp32)
            W21 = sm.tile([P, T], fp32)
            sr1 = sm.tile([P, T], fp32)
            sr2 = sm.tile([P, T], fp32)
            sd1 = sm.tile([P, T], fp32)
            sd2 = sm.tile([P, T], fp32)
            t12a = sm.tile([P, T], fp32)
            t12b = sm.tile([P, T], fp32)
            t21a = sm.tile([P, T], fp32)
            t21b = sm.tile([P, T], fp32)
            outt = sm.tile([P, 3 * T], fp32)

            # primer: a no-dependency ACT instruction so the ~1.3us ACT table
            # load (inserted before the first activation in the scheduled
            # stream) runs at t=0 instead of behind the first input DMA
            dm = sm.tile([P, 1], fp32)
            dmo = sm.tile([P, 1], fp32)
            nc.vector.memset(dm[:], 0.0)
            nc.scalar.activation(dmo[:], dm[:], AF.Exp)

            def emit_expU_M(t, ut, split=False):
                # sigmoid(x) = exp(u) with u = log(sigmoid(x)) — stays in the
                # exp/ln ACT table set. M12 = sum sg1*log(sg2), M21 symmetric.
                # split=True emits the exp per half so each M product starts
                # as soon as its own sigmoid half lands (shrinks the tail for
                # the last tile, whose products trail the final ACT pass).
                sgt = sgp.tile([P, 2 * C], bf16, tag="sg")
                if not split:
                    nc.scalar.activation(sgt[:], ut[:], AF.Exp)
                else:
                    nc.scalar.activation(sgt[:, 0:C], ut[:, 0:C], AF.Exp)
                sc2 = scp.tile([P, 2 * C], bf16, tag="sc")
                nc.vector.scalar_tensor_tensor(
                    sc2[:, 0:C], sgt[:, 0:C], 1.0, ut[:, C : 2 * C],
                    op0=ALU.mult, op1=ALU.mult,
                    accum_out=outt[:, T + t : T + t + 1],
                )
                if split:
                    nc.scalar.activation(sgt[:, C : 2 * C], ut[:, C : 2 * C], AF.Exp)
                nc.vector.scalar_tensor_tensor(
                    sc2[:, C : 2 * C], sgt[:, C : 2 * C], 1.0, ut[:, 0:C],
                    op0=ALU.mult, op1=ALU.mult,
                    accum_out=outt[:, 2 * T + t : 2 * T + t + 1],
                )

            uts = []
            for t in range(T):
                r0, r1 = t * P, (t + 1) * P
                # two DMA queues (sync HWDGE + gpsimd SWDGE) so the halves
                # land in parallel
                if t == 0:
                    # tile 0 on two separate tiles: per-tensor deps then let
                    # exp of the x1 half start as soon as its own DMA lands
                    xta = xp.tile([P, C], fp32, tag="xa")
                    xtb = xp.tile([P, C], fp32, tag="xb")
                    nc.sync.dma_start(xtb[:], x2d[r0:r1, :])
                    nc.sync.dma_start(xta[:], x1d[r0:r1, :])
                    nc.sync.dma_start(gt[:, 0:TK], g1d)
                    nc.sync.dma_start(gt[:, TK : 2 * TK], g2d)
                    xparts = [(xtb, slice(C, 2 * C)), (xta, slice(0, C))]
                else:
                    xt = xp.tile([P, 2 * C], fp32, tag="x")
                    nc.sync.dma_start(xt[:, 0:C], x1d[r0:r1, :])
                    nc.sync.dma_start(xt[:, C : 2 * C], x2d[r0:r1, :])
                    xparts = [(xt, slice(0, 2 * C))]

                At = ap_.tile([P, 2 * C], bf16, tag="A")
                for xsrc, dsl in xparts:
                    nc.scalar.activation(At[:, dsl], xsrc[:], AF.Exp)
                nc.vector.tensor_reduce(
                    E1t[:, t : t + 1], At[:, 0:C], axis=AX.X, op=ALU.add
                )
                nc.vector.tensor_reduce(
                    E2t[:, t : t + 1], At[:, C : 2 * C], axis=AX.X, op=ALU.add
                )

                if t == 0:
                    nc.scalar.activation(aa[:], gt[:], AF.Exp)
                    nc.vector.tensor_reduce(
                        P1t[:], aa[:, 0:TK].rearrange("p (t k) -> p t k", k=K),
                        axis=AX.X, op=ALU.add,
                    )
                    nc.vector.tensor_reduce(
                        P2t[:], aa[:, TK : 2 * TK].rearrange("p (t k) -> p t k", k=K),
                        axis=AX.X, op=ALU.add,
                    )
                    nc.vector.tensor_scalar_mul(P1s[:], P1t[:], EPS * (K - 1) / K)
                    nc.vector.tensor_scalar_mul(P2s[:], P2t[:], EPS * (K - 1) / K)

                # per-row scalars for this tile: tbar = eps*E - eps*(K-1)/K*P
                tt = slice(t, t + 1)
                nc.vector.scalar_tensor_tensor(
                    tb1[:, tt], E1t[:, tt], EPS, P1s[:, tt],
                    op0=ALU.mult, op1=ALU.subtract,
                )
                nc.vector.scalar_tensor_tensor(
                    tb2[:, tt], E2t[:, tt], EPS, P2s[:, tt],
                    op0=ALU.mult, op1=ALU.subtract,
                )
                nc.vector.tensor_sub(E1n[:, tt], E1t[:, tt], P1t[:, tt])
                nc.vector.tensor_sub(E2n[:, tt], E2t[:, tt], P2t[:, tt])

                # SM fragments for this tile: [a1+tb1 | a2+tb2 | D1 | D2]
                c0 = t * K
                nc.vector.tensor_scalar(
                    SM[:, c0 : c0 + K], aa[:, c0 : c0 + K],
                    tb1[:, t : t + 1], None, op0=ALU.add,
                )
                nc.vector.tensor_scalar(
                    SM[:, TK + c0 : TK + c0 + K], aa[:, TK + c0 : TK + c0 + K],
                    tb2[:, t : t + 1], None, op0=ALU.add,
                )
                nc.vector.tensor_scalar(
                    SM[:, 2 * TK + c0 : 2 * TK + c0 + K], aa[:, c0 : c0 + K],
                    E1n[:, t : t + 1], None, op0=ALU.add,
                )
                nc.vector.tensor_scalar(
                    SM[:, 3 * TK + c0 : 3 * TK + c0 + K],
                    aa[:, TK + c0 : TK + c0 + K],
                    E2n[:, t : t + 1], None, op0=ALU.add,
                )

                # ln(A+1) = softplus(x); u = x - ln(1+A) = log(sigmoid(x)),
                # computed on the otherwise-idle gpsimd engine
                LLpt = llpp.tile([P, 2 * C], fp32, tag="llp")
                nc.scalar.activation(LLpt[:], At[:], AF.Ln, bias=1.0)
                ut = up.tile([P, 2 * C], bf16, tag="u")
                for xsrc, dsl in xparts:
                    nc.gpsimd.tensor_sub(ut[:, dsl], xsrc[:], LLpt[:, dsl])
                uts.append(ut)

                # LL = ln(A + tbar); L12 = sum A1*LL2, L21 = sum A2*LL1
                LLt = llp.tile([P, 2 * C], bf16, tag="ll")
                nc.scalar.activation(
                    LLt[:, 0:C], At[:, 0:C], AF.Ln, bias=tb1[:, t : t + 1]
                )
                nc.scalar.activation(
                    LLt[:, C : 2 * C], At[:, C : 2 * C], AF.Ln,
                    bias=tb2[:, t : t + 1],
                )
                sc = scp.tile([P, 2 * C], bf16, tag="sc")
                nc.vector.scalar_tensor_tensor(
                    sc[:, 0:C], At[:, 0:C], 1.0, LLt[:, C : 2 * C],
                    op0=ALU.mult, op1=ALU.mult, accum_out=Lt[:, t : t + 1],
                )
                nc.vector.scalar_tensor_tensor(
                    sc[:, C : 2 * C], At[:, C : 2 * C], 1.0, LLt[:, 0:C],
                    op0=ALU.mult, op1=ALU.mult,
                    accum_out=Lt[:, T + t : T + t + 1],
                )

                if t < T - 1:
                    emit_expU_M(t, ut)

            # ---- small assembly: row_single per (row, tile) ----
            nc.scalar.activation(LG[:], SM[:], AF.Ln)
            nc.vector.reciprocal(REC[:], SM[:, 2 * TK : 4 * TK])

            lga1, lga2 = LG[:, 0:TK], LG[:, TK : 2 * TK]
            lgD1, lgD2 = LG[:, 2 * TK : 3 * TK], LG[:, 3 * TK : 4 * TK]
            rec1, rec2 = REC[:, 0:TK], REC[:, TK : 2 * TK]
            nc.vector.tensor_mul(u12[:], aa[:, 0:TK], lga2)
            nc.vector.tensor_mul(u21[:], aa[:, TK : 2 * TK], lga1)
            nc.vector.tensor_mul(w12[:], rec1, u12[:])
            nc.vector.tensor_mul(w21[:], rec2, u21[:])
            grp = lambda apx: apx.rearrange("p (t k) -> p t k", k=K)
            nc.vector.tensor_reduce(S12[:], grp(u12[:]), axis=AX.X, op=ALU.add)
            nc.vector.tensor_reduce(S21[:], grp(u21[:]), axis=AX.X, op=ALU.add)
            nc.vector.tensor_reduce(W12[:], grp(w12[:]), axis=AX.X, op=ALU.add)
            nc.vector.tensor_reduce(W21[:], grp(w21[:]), axis=AX.X, op=ALU.add)
            nc.vector.tensor_reduce(sr1[:], grp(rec1), axis=AX.X, op=ALU.add)
            nc.vector.tensor_reduce(sr2[:], grp(rec2), axis=AX.X, op=ALU.add)
            nc.vector.tensor_reduce(sd1[:], grp(lgD1), axis=AX.X, op=ALU.add)
            nc.vector.tensor_reduce(sd2[:], grp(lgD2), axis=AX.X, op=ALU.add)

            # row_single = sd2 - (L12-S12)*sr1 - W12 + sd1 - (L21-S21)*sr2 - W21
            nc.vector.tensor_sub(t12a[:], Lt[:, 0:T], S12[:])
            nc.vector.tensor_mul(t12b[:], t12a[:], sr1[:])
            nc.vector.tensor_sub(t21a[:], Lt[:, T : 2 * T], S21[:])
            nc.vector.tensor_mul(t21b[:], t21a[:], sr2[:])
            nc.vector.tensor_add(t12a[:], sd1[:], sd2[:])
            nc.vector.tensor_sub(t12a[:], t12a[:], t12b[:])
            nc.vector.tensor_sub(t12a[:], t12a[:], t21b[:])
            nc.vector.tensor_sub(t12a[:], t12a[:], W12[:])
            nc.vector.tensor_sub(outt[:, 0:T], t12a[:], W21[:])

            # last tile's sigmoid chain emitted after the assembly so the only
            # post-ACT work is its two M products + the output DMA
            emit_expU_M(T - 1, uts[T - 1], split=True)

            nc.sync.dma_start(outd, outt[:])

    nc.compile()
    return nc


def _get_nc():
    if "nc" not in _cache:
        _cache["nc"] = _build()
    return _cache["nc"]


def kernel(out1, out2, para, target, pos_idx):
    from concourse.bass_utils import run_bass_kernel_spmd

    nc = _get_nc()

    out1 = np.ascontiguousarray(out1, dtype=np.float32)
    out2 = np.ascontiguousarray(out2, dtype=np.float32)
    idx = pos_idx.astype(np.int64)
    g1 = np.take_along_axis(out1, idx, axis=1)   # [B, K]
    g2 = np.take_along_axis(out2, idx, axis=1)

    def pack(g, c):
        # [RPC, K] -> [P, T*K] with col t*K+k = row (t*P + p)
        s = g[c * RPC : (c + 1) * RPC]
        return np.ascontiguousarray(
            s.reshape(T, P, K).transpose(1, 0, 2).reshape(P, TK)
        )

    in_maps = [
        {
            "x1": out1[c * RPC : (c + 1) * RPC],
            "x2": out2[c * RPC : (c + 1) * RPC],
            "g1": pack(g1, c),
            "g2": pack(g2, c),
        }
        for c in range(NCORES)
    ]
    res = run_bass_kernel_spmd(nc, in_maps, core_ids=list(range(NCORES)))
    parts = np.stack([r["out"] for r in res.results])  # [NCORES, P, 3T]

    single = parts[:, :, 0:T].sum(dtype=np.float64) / (B * K)
    multi = -parts[:, :, T : 3 * T].sum(dtype=np.float64) / B
    p = float(np.asarray(para))
    return np.asarray(p * multi + (1.0 - p) * single, dtype=np.float32)



# revision 3
# speedup vs baseline: 1.3559x; 1.3559x over previous
"""Trainium2 Bass kernel for nn_DUDCLoss_1382979469646.

Data-parallel over the batch dim: 8 cores x 512 rows each. The loss is
factorized so each row needs only a handful of C-length passes.

Single (masked-softmax) part: with A=exp(x), E=sum(A), a_k=A[pos_k],
En=E-sum_k(a_k), D_j=En+a_j, the per-(row,j) cross-entropy is
  xent12_j = ln(D2_j) - (G12 - S12 + a1_j*g2_j) / D1_j
where G12 = sum_c A1_c * x2_c and S12 = sum_k a1_k * g2_k. This uses
ln(p+eps) ~= ln(p) (drops the +eps inside the log); measured total error
1.3e-3 relative - well inside the 2e-2 gate. Eliminating the eps terms
removes the ln(A+tbar) ACT pass of the previous version entirely.

Multi (sigmoid) part: ln(sigmoid(x)+eps) ~= u = x - ln(1+A), and
sigmoid(x) = exp(u), so every transcendental stays in the one ACT table
set that holds both Exp and Ln (a patched table-selection policy
guarantees a single ~1.3us table load). The u subtraction runs on the
otherwise-idle gpsimd engine.

Engine budget per core (cost model): ACT 3 passes over 2C*T elems
~20.5us (the bottleneck); DVE products as tensor_tensor mult (2-byte
dtypes, 2x mode) + tensor_scalar accumulate (4x mode) ~13us; gpsimd
~11us; DMA ~6us (inputs converted to fp16 on host - also keeps every
DVE operand 2-byte for the fast modes).

Each core writes [128, 2T] partial sums; the host does the final tiny
reduction and the para blend.
"""

import numpy as np

NCORES = 8
B, C, K = 4096, 1024, 8
RPC = B // NCORES          # rows per core
P = 128                    # partitions
T = RPC // P               # row-tiles per core
TK = T * K
EPS = 1e-5

_cache = {}


def _patch_act_tables(mybir, bacc):
    """Make the ACT-table-load inserter resolve both Exp and Ln to the one
    set that holds both (natural_log_exp_and_others). The default policy
    picks a singleton set per function, inserting a ~1.3us table load at
    every Exp<->Ln transition in the scheduled stream."""
    if getattr(bacc, "_dudc_act_patch", False):
        return
    orig = bacc.get_activation_tables
    both = {mybir.ActivationFunctionType.Exp, mybir.ActivationFunctionType.Ln}

    def patched(arch):
        tabs = orig(arch)
        if any(both <= funcs for funcs in tabs.values()):
            for name, funcs in tabs.items():
                if not both <= funcs:
                    funcs.difference_update(both)
        return tabs

    bacc.get_activation_tables = patched
    bacc._dudc_act_patch = True


def _build():
    import concourse.bass as bass
    import concourse.tile as tile
    from concourse import bacc, mybir

    _patch_act_tables(mybir, bacc)

    fp32 = mybir.dt.float32
    fp16 = mybir.dt.float16
    AF = mybir.ActivationFunctionType
    ALU = mybir.AluOpType
    AX = mybir.AxisListType

    nc = bacc.Bacc(
        "TRN2",
        target_bir_lowering=False,
        debug=False,
        num_devices=NCORES,
    )

    x1d = nc.dram_tensor("x1", [RPC, C], fp16, kind="ExternalInput").ap()
    x2d = nc.dram_tensor("x2", [RPC, C], fp16, kind="ExternalInput").ap()
    g1d = nc.dram_tensor("g1", [P, TK], fp32, kind="ExternalInput").ap()
    g2d = nc.dram_tensor("g2", [P, TK], fp32, kind="ExternalInput").ap()
    outd = nc.dram_tensor("out", [P, 2 * T], fp32, kind="ExternalOutput").ap()

    with tile.TileContext(nc) as tc:
        with (
            tc.tile_pool(name="x", bufs=3) as xp,
            tc.tile_pool(name="A", bufs=2) as ap_,
            tc.tile_pool(name="llp", bufs=2) as llpp,
            tc.tile_pool(name="u", bufs=2) as up,
            tc.tile_pool(name="sg", bufs=2) as sgp,
            tc.tile_pool(name="q", bufs=2) as qp,
            tc.tile_pool(name="scratch", bufs=4) as scp,
            tc.tile_pool(name="small", bufs=1) as sm,
        ):
            # ---- persistent small tiles ----
            gt = sm.tile([P, 2 * TK], fp32)        # g1 | g2
            aa = sm.tile([P, 2 * TK], fp32)        # exp(g1) | exp(g2)
            E1t = sm.tile([P, T], fp32)
            E2t = sm.tile([P, T], fp32)
            P1t = sm.tile([P, T], fp32)
            P2t = sm.tile([P, T], fp32)
            E1n = sm.tile([P, T], fp32)
            E2n = sm.tile([P, T], fp32)
            G12 = sm.tile([P, T], fp32)
            G21 = sm.tile([P, T], fp32)
            SM = sm.tile([P, 2 * TK], fp32)        # D1 | D2
            LG = sm.tile([P, 2 * TK], fp32)        # ln of SM
            REC = sm.tile([P, 2 * TK], fp32)       # 1/D1 | 1/D2
            u12 = sm.tile([P, TK], fp32)
            u21 = sm.tile([P, TK], fp32)
            w12 = sm.tile([P, TK], fp32)
            w21 = sm.tile([P, TK], fp32)
            S12 = sm.tile([P, T], fp32)
            S21 = sm.tile([P, T], fp32)
            W12 = sm.tile([P, T], fp32)
            W21 = sm.tile([P, T], fp32)
            sr1 = sm.tile([P, T], fp32)
            sr2 = sm.tile([P, T], fp32)
            sd1 = sm.tile([P, T], fp32)
            sd2 = sm.tile([P, T], fp32)
            t12a = sm.tile([P, T], fp32)
            t12b = sm.tile([P, T], fp32)
            t21a = sm.tile([P, T], fp32)
            t21b = sm.tile([P, T], fp32)
            outt = sm.tile([P, 2 * T], fp32)

            # primer: a no-dependency ACT instruction so the ~1.3us ACT table
            # load (inserted before the first activation in the scheduled
            # stream) runs at t=0 instead of behind the first input DMA
            dm = sm.tile([P, 1], fp32)
            dmo = sm.tile([P, 1], fp32)
            nc.vector.memset(dm[:], 0.0)
            nc.scalar.activation(dmo[:], dm[:], AF.Exp)

            def emit_sig_M(t, ut, split=False):
                # sigmoid(x) = exp(u) with u = log(sigmoid(x)). The combined
                # M12+M21 = sum s1*u2 + s2*u1 goes into one accumulator
                # (only the total enters the loss). split=True emits exp per
                # half so each M product starts as soon as its half lands
                # (shrinks the last tile's tail).
                sgt = sgp.tile([P, 2 * C], fp16, tag="sg")
                if not split:
                    nc.scalar.activation(sgt[:], ut[:], AF.Exp)
                else:
                    nc.scalar.activation(sgt[:, 0:C], ut[:, 0:C], AF.Exp)
                qm = qp.tile([P, 2 * C], fp16, tag="qm")
                sc = scp.tile([P, 2 * C], fp16, tag="sc")
                nc.vector.tensor_tensor(
                    qm[:, 0:C], sgt[:, 0:C], ut[:, C : 2 * C], op=ALU.mult
                )
                if split:
                    nc.scalar.activation(sgt[:, C : 2 * C], ut[:, C : 2 * C], AF.Exp)
                nc.vector.tensor_tensor(
                    qm[:, C : 2 * C], sgt[:, C : 2 * C], ut[:, 0:C], op=ALU.mult
                )
                if not split:
                    nc.vector.tensor_scalar(
                        sc[:], qm[:], 1.0, 0.0, op0=ALU.mult, op1=ALU.add,
                        accum_out=outt[:, T + t : T + t + 1],
                    )
                else:
                    # per-half accums into separate outt columns would clash;
                    # accumulate the two halves into the same column via two
                    # instructions is not supported (accum overwrites), so
                    # split mode sums halves into scratch accums combined by
                    # the final assembly. Simpler: one accum after both qm
                    # halves (the mults are what we want started early).
                    nc.vector.tensor_scalar(
                        sc[:], qm[:], 1.0, 0.0, op0=ALU.mult, op1=ALU.add,
                        accum_out=outt[:, T + t : T + t + 1],
                    )

            uts = []
            for t in range(T):
                r0, r1 = t * P, (t + 1) * P
                if t == 0:
                    # tile 0 on two separate tiles: per-tensor deps so exp of
                    # the x2 half starts as soon as its own DMA lands
                    xta = xp.tile([P, C], fp16, tag="xa")
                    xtb = xp.tile([P, C], fp16, tag="xb")
                    nc.sync.dma_start(xtb[:], x2d[r0:r1, :])
                    nc.sync.dma_start(xta[:], x1d[r0:r1, :])
                    nc.sync.dma_start(gt[:, 0:TK], g1d)
                    nc.sync.dma_start(gt[:, TK : 2 * TK], g2d)
                    xparts = [(xtb, slice(C, 2 * C)), (xta, slice(0, C))]
                    x1v, x2v = xta[:], xtb[:]
                else:
                    xt = xp.tile([P, 2 * C], fp16, tag="x")
                    nc.sync.dma_start(xt[:, 0:C], x1d[r0:r1, :])
                    nc.sync.dma_start(xt[:, C : 2 * C], x2d[r0:r1, :])
                    xparts = [(xt, slice(0, 2 * C))]
                    x1v, x2v = xt[:, 0:C], xt[:, C : 2 * C]

                At = ap_.tile([P, 2 * C], fp16, tag="A")
                for xsrc, dsl in xparts:
                    nc.scalar.activation(At[:, dsl], xsrc[:], AF.Exp)

                # row sums E via fast tensor_scalar accumulate (4x mode)
                sce = scp.tile([P, 2 * C], fp16, tag="sce")
                nc.vector.tensor_scalar(
                    sce[:, 0:C], At[:, 0:C], 1.0, 0.0, op0=ALU.mult, op1=ALU.add,
                    accum_out=E1t[:, t : t + 1],
                )
                nc.vector.tensor_scalar(
                    sce[:, C : 2 * C], At[:, C : 2 * C], 1.0, 0.0, op0=ALU.mult, op1=ALU.add,
                    accum_out=E2t[:, t : t + 1],
                )

                # cross products G12 = sum A1*x2, G21 = sum A2*x1
                qg = qp.tile([P, 2 * C], fp16, tag="qg")
                nc.vector.tensor_tensor(qg[:, 0:C], At[:, 0:C], x2v, op=ALU.mult)
                nc.vector.tensor_tensor(
                    qg[:, C : 2 * C], At[:, C : 2 * C], x1v, op=ALU.mult
                )
                scg = scp.tile([P, 2 * C], fp16, tag="scg")
                nc.vector.tensor_scalar(
                    scg[:, 0:C], qg[:, 0:C], 1.0, 0.0, op0=ALU.mult, op1=ALU.add,
                    accum_out=G12[:, t : t + 1],
                )
                nc.vector.tensor_scalar(
                    scg[:, C : 2 * C], qg[:, C : 2 * C], 1.0, 0.0, op0=ALU.mult, op1=ALU.add,
                    accum_out=G21[:, t : t + 1],
                )

                if t == 0:
                    nc.scalar.activation(aa[:], gt[:], AF.Exp)
                    nc.vector.tensor_reduce(
                        P1t[:], aa[:, 0:TK].rearrange("p (t k) -> p t k", k=K),
                        axis=AX.X, op=ALU.add,
                    )
                    nc.vector.tensor_reduce(
                        P2t[:], aa[:, TK : 2 * TK].rearrange("p (t k) -> p t k", k=K),
                        axis=AX.X, op=ALU.add,
                    )

                # En = E - sum_k a_k, then D fragments [D1 | D2]
                tt = slice(t, t + 1)
                nc.vector.tensor_sub(E1n[:, tt], E1t[:, tt], P1t[:, tt])
                nc.vector.tensor_sub(E2n[:, tt], E2t[:, tt], P2t[:, tt])
                c0 = t * K
                nc.vector.tensor_scalar(
                    SM[:, c0 : c0 + K], aa[:, c0 : c0 + K],
                    E1n[:, t : t + 1], None, op0=ALU.add,
                )
                nc.vector.tensor_scalar(
                    SM[:, TK + c0 : TK + c0 + K], aa[:, TK + c0 : TK + c0 + K],
                    E2n[:, t : t + 1], None, op0=ALU.add,
                )

                # u = x - ln(1+A) = log(sigmoid(x)) on the gpsimd engine
                LLpt = llpp.tile([P, 2 * C], fp32, tag="llp")
                nc.scalar.activation(LLpt[:], At[:], AF.Ln, bias=1.0)
                ut = up.tile([P, 2 * C], fp16, tag="u")
                for xsrc, dsl in xparts:
                    nc.gpsimd.tensor_sub(ut[:, dsl], xsrc[:], LLpt[:, dsl])
                uts.append(ut)

                if t < T - 1:
                    emit_sig_M(t, ut)

            # ---- small assembly: row_single per (row, tile) ----
            nc.scalar.activation(LG[:], SM[:], AF.Ln)
            nc.vector.reciprocal(REC[:], SM[:])

            rec1, rec2 = REC[:, 0:TK], REC[:, TK : 2 * TK]
            lgD1, lgD2 = LG[:, 0:TK], LG[:, TK : 2 * TK]
            # S12 = sum_k a1*g2 (positive-label correction), W12 = sum_k a1*g2/D1
            nc.vector.tensor_mul(u12[:], aa[:, 0:TK], gt[:, TK : 2 * TK])
            nc.vector.tensor_mul(u21[:], aa[:, TK : 2 * TK], gt[:, 0:TK])
            nc.vector.tensor_mul(w12[:], rec1, u12[:])
            nc.vector.tensor_mul(w21[:], rec2, u21[:])
            grp = lambda apx: apx.rearrange("p (t k) -> p t k", k=K)
            nc.vector.tensor_reduce(S12[:], grp(u12[:]), axis=AX.X, op=ALU.add)
            nc.vector.tensor_reduce(S21[:], grp(u21[:]), axis=AX.X, op=ALU.add)
            nc.vector.tensor_reduce(W12[:], grp(w12[:]), axis=AX.X, op=ALU.add)
            nc.vector.tensor_reduce(W21[:], grp(w21[:]), axis=AX.X, op=ALU.add)
            nc.vector.tensor_reduce(sr1[:], grp(rec1), axis=AX.X, op=ALU.add)
            nc.vector.tensor_reduce(sr2[:], grp(rec2), axis=AX.X, op=ALU.add)
            nc.vector.tensor_reduce(sd1[:], grp(lgD1), axis=AX.X, op=ALU.add)
            nc.vector.tensor_reduce(sd2[:], grp(lgD2), axis=AX.X, op=ALU.add)

            # row_single = sd2 - (G12-S12)*sr1 - W12 + sd1 - (G21-S21)*sr2 - W21
            nc.vector.tensor_sub(t12a[:], G12[:], S12[:])
            nc.vector.tensor_mul(t12b[:], t12a[:], sr1[:])
            nc.vector.tensor_sub(t21a[:], G21[:], S21[:])
            nc.vector.tensor_mul(t21b[:], t21a[:], sr2[:])
            nc.vector.tensor_add(t12a[:], sd1[:], sd2[:])
            nc.vector.tensor_sub(t12a[:], t12a[:], t12b[:])
            nc.vector.tensor_sub(t12a[:], t12a[:], t21b[:])
            nc.vector.tensor_sub(t12a[:], t12a[:], W12[:])
            nc.vector.tensor_sub(outt[:, 0:T], t12a[:], W21[:])

            # last tile's sigmoid chain emitted after the assembly so the only
            # post-ACT work is its two M products + the output DMA
            emit_sig_M(T - 1, uts[T - 1], split=True)

            nc.sync.dma_start(outd, outt[:])

    nc.compile()
    return nc


def _get_nc():
    if "nc" not in _cache:
        _cache["nc"] = _build()
    return _cache["nc"]


def kernel(out1, out2, para, target, pos_idx):
    from concourse.bass_utils import run_bass_kernel_spmd

    nc = _get_nc()

    out1 = np.ascontiguousarray(out1, dtype=np.float32)
    out2 = np.ascontiguousarray(out2, dtype=np.float32)
    x1h = out1.astype(np.float16)
    x2h = out2.astype(np.float16)
    idx = pos_idx.astype(np.int64)
    g1 = np.take_along_axis(out1, idx, axis=1)   # [B, K]
    g2 = np.take_along_axis(out2, idx, axis=1)

    def pack(g, c):
        # [RPC, K] -> [P, T*K] with col t*K+k = row (t*P + p)
        s = g[c * RPC : (c + 1) * RPC]
        return np.ascontiguousarray(
            s.reshape(T, P, K).transpose(1, 0, 2).reshape(P, TK)
        )

    in_maps = [
        {
            "x1": x1h[c * RPC : (c + 1) * RPC],
            "x2": x2h[c * RPC : (c + 1) * RPC],
            "g1": pack(g1, c),
            "g2": pack(g2, c),
        }
        for c in range(NCORES)
    ]
    res = run_bass_kernel_spmd(nc, in_maps, core_ids=list(range(NCORES)))
    parts = np.stack([r["out"] for r in res.results])  # [NCORES, P, 2T]

    single = parts[:, :, 0:T].sum(dtype=np.float64) / (B * K)
    multi = -parts[:, :, T : 2 * T].sum(dtype=np.float64) / B
    p = float(np.asarray(para))
    return np.asarray(p * multi + (1.0 - p) * single, dtype=np.float32)


# revision 5
# speedup vs baseline: 1.4070x; 1.0376x over previous
"""Trainium2 Bass kernel for nn_DUDCLoss_1382979469646.

Data-parallel over the batch dim: 8 cores x 512 rows each. The loss is
factorized so each row needs only a handful of C-length passes; the
device computes exactly five per-row reductions and the host finishes
the tiny O(B*K) assembly in fp64 (mirroring the host-side gather the
input already requires).

Single (masked-softmax) part: with A=exp(x), E=sum(A), a_k=A[pos_k],
En=E-sum_k(a_k), D_j=En+a_j, the per-(row,j) cross-entropy is
  xent12_j = ln(D2_j) - (G12 - S12 + a1_j*g2_j) / D1_j
where G12 = sum_c A1_c * x2_c and S12 = sum_k a1_k * g2_k. This uses
ln(p+eps) ~= ln(p) (drops the +eps inside the log); measured total error
~1.3e-3 relative - well inside the 2e-2 gate. Dropping eps makes the
log of the softmax numerator the input logit itself, so no ln pass is
needed for the cross terms. Device ships E1,E2,G12,G21 per row; host
(which already gathered g = x[pos]) forms D, ln(D), 1/D over [B,K].

Multi (sigmoid) part: ln(sigmoid(x)+eps) ~= u = x - ln(1+A) and
sigmoid(x) = exp(u); both ACT functions (Exp, Ln) live in the one ACT
table set that holds both (a patched table-selection policy guarantees
a single ~1.3us table load). Device ships the accumulated
sum(s1*u2 + s2*u1) per row-tile.

Engine split per core (cost model): ACT 3 passes (exp, ln1p, exp(u))
~23us - the bottleneck and within ~15% of the 4-engine work lower
bound; Pool (gpsimd) takes the G products and the u subtraction ~14us;
DVE takes the row accumulators (tensor_scalar accum, 4x mode) and the
multi products ~13us; DMA ~6us (fp16 inputs, converted on host - fp16
also keeps DVE operands 2-byte for the fast modes). The last tile's
exp(u) is emitted in quarters so its products/accums/output DMA trail
the ACT stream by only ~2.5us.
"""

import numpy as np

NCORES = 8
B, C, K = 4096, 1024, 8
RPC = B // NCORES          # rows per core
P = 128                    # partitions
T = RPC // P               # row-tiles per core
TK = T * K
EPS = 1e-5
NQ = 4                     # last-tile exp(u) quarter splits
OUTW = 4 * T + (T - 1) + NQ  # E1,E2,G12,G21 | M per tile | M quarters

_cache = {}


def _patch_act_tables(mybir, bacc):
    """Make the ACT-table-load inserter resolve both Exp and Ln to the one
    set that holds both (natural_log_exp_and_others). The default policy
    picks a singleton set per function, inserting a ~1.3us table load at
    every Exp<->Ln transition in the scheduled stream."""
    if getattr(bacc, "_dudc_act_patch", False):
        return
    orig = bacc.get_activation_tables
    both = {mybir.ActivationFunctionType.Exp, mybir.ActivationFunctionType.Ln}

    def patched(arch):
        tabs = orig(arch)
        if any(both <= funcs for funcs in tabs.values()):
            for name, funcs in tabs.items():
                if not both <= funcs:
                    funcs.difference_update(both)
        return tabs

    bacc.get_activation_tables = patched
    bacc._dudc_act_patch = True


def _build():
    import concourse.bass as bass
    import concourse.tile as tile
    from concourse import bacc, mybir

    _patch_act_tables(mybir, bacc)

    fp32 = mybir.dt.float32
    fp16 = mybir.dt.float16
    AF = mybir.ActivationFunctionType
    ALU = mybir.AluOpType

    nc = bacc.Bacc(
        "TRN2",
        target_bir_lowering=False,
        debug=False,
        num_devices=NCORES,
    )

    x1d = nc.dram_tensor("x1", [RPC, C], fp16, kind="ExternalInput").ap()
    x2d = nc.dram_tensor("x2", [RPC, C], fp16, kind="ExternalInput").ap()
    outd = nc.dram_tensor("out", [P, OUTW], fp32, kind="ExternalOutput").ap()

    cE1, cE2, cG12, cG21, cM = 0, T, 2 * T, 3 * T, 4 * T

    with tile.TileContext(nc) as tc:
        with (
            tc.tile_pool(name="x", bufs=3) as xp,
            tc.tile_pool(name="A", bufs=2) as ap_,
            tc.tile_pool(name="llp", bufs=2) as llpp,
            tc.tile_pool(name="u", bufs=2) as up,
            tc.tile_pool(name="sg", bufs=2) as sgp,
            tc.tile_pool(name="q", bufs=3) as qp,
            tc.tile_pool(name="scratch", bufs=4) as scp,
            tc.tile_pool(name="small", bufs=1) as sm,
        ):
            outt = sm.tile([P, OUTW], fp32)

            # primer: a no-dependency ACT instruction so the ~1.3us ACT table
            # load (inserted before the first activation in the scheduled
            # stream) runs at t=0 instead of behind the first input DMA
            dm = sm.tile([P, 1], fp32)
            dmo = sm.tile([P, 1], fp32)
            nc.vector.memset(dm[:], 0.0)
            nc.scalar.activation(dmo[:], dm[:], AF.Exp)

            def acc(dst_col, src_ap):
                # fast row accumulate: tensor_scalar keeps 4x perf mode
                scw = scp.tile([P, 2 * C], fp16, tag="scw")
                w = src_ap.shape[-1]
                nc.vector.tensor_scalar(
                    scw[:, 0:w], src_ap, 1.0, 0.0, op0=ALU.mult, op1=ALU.add,
                    accum_out=outt[:, dst_col : dst_col + 1],
                )

            def emit_sig_M(t, ut):
                # s = exp(u); M accum = sum s1*u2 + s2*u1 (combined: only the
                # total enters the loss)
                sgt = sgp.tile([P, 2 * C], fp16, tag="sg")
                nc.scalar.activation(sgt[:], ut[:], AF.Exp)
                qm = qp.tile([P, 2 * C], fp16, tag="qm")
                nc.vector.tensor_mul(qm[:, 0:C], sgt[:, 0:C], ut[:, C : 2 * C])
                nc.vector.tensor_mul(qm[:, C : 2 * C], sgt[:, C : 2 * C], ut[:, 0:C])
                acc(cM + t, qm[:])

            uts = []
            for t in range(T):
                r0, r1 = t * P, (t + 1) * P
                if t == 0:
                    # tile 0 on two separate tiles: per-tensor deps so exp of
                    # the x2 half starts as soon as its own DMA lands
                    xta = xp.tile([P, C], fp16, tag="xa")
                    xtb = xp.tile([P, C], fp16, tag="xb")
                    nc.sync.dma_start(xtb[:], x2d[r0:r1, :])
                    nc.sync.dma_start(xta[:], x1d[r0:r1, :])
                    xparts = [(xtb, slice(C, 2 * C)), (xta, slice(0, C))]
                    x1v, x2v = xta[:], xtb[:]
                else:
                    xt = xp.tile([P, 2 * C], fp16, tag="x")
                    nc.sync.dma_start(xt[:, 0:C], x1d[r0:r1, :])
                    nc.sync.dma_start(xt[:, C : 2 * C], x2d[r0:r1, :])
                    xparts = [(xt, slice(0, 2 * C))]
                    x1v, x2v = xt[:, 0:C], xt[:, C : 2 * C]

                At = ap_.tile([P, 2 * C], fp16, tag="A")
                for xsrc, dsl in xparts:
                    nc.scalar.activation(At[:, dsl], xsrc[:], AF.Exp)

                # row sums E1, E2 straight into out columns
                acc(cE1 + t, At[:, 0:C])
                acc(cE2 + t, At[:, C : 2 * C])

                # cross products on the gpsimd engine: G12 = sum A1*x2 etc.
                qg = qp.tile([P, 2 * C], fp16, tag="qg")
                nc.gpsimd.tensor_tensor(qg[:, 0:C], At[:, 0:C], x2v, op=ALU.mult)
                nc.gpsimd.tensor_tensor(
                    qg[:, C : 2 * C], At[:, C : 2 * C], x1v, op=ALU.mult
                )
                acc(cG12 + t, qg[:, 0:C])
                acc(cG21 + t, qg[:, C : 2 * C])

                # u = x - ln(1+A) = log(sigmoid(x)), subtract on gpsimd
                LLpt = llpp.tile([P, 2 * C], fp32, tag="llp")
                nc.scalar.activation(LLpt[:], At[:], AF.Ln, bias=1.0)
                ut = up.tile([P, 2 * C], fp16, tag="u")
                if t == 0 or t == T - 1:
                    # halves: t0 for per-tensor deps, t3 so the first exp(u)
                    # quarter isn't gated on the full subtract
                    for xsrc, dsl in xparts:
                        if t == 0:
                            nc.gpsimd.tensor_sub(ut[:, dsl], xsrc[:], LLpt[:, dsl])
                        else:
                            for h in (slice(0, C), slice(C, 2 * C)):
                                nc.gpsimd.tensor_sub(ut[:, h], xt[:, h], LLpt[:, h])
                            break
                else:
                    nc.gpsimd.tensor_sub(ut[:], xt[:], LLpt[:])
                uts.append(ut)

                # previous tile's sigmoid chain lands here so its exp(u) runs
                # after this tile's exp/ln1p on ACT (u is ready by then)
                if t >= 1:
                    emit_sig_M(t - 1, uts[t - 1])

            # last tile's sigmoid chain in quarters: each exp(u) quarter is
            # followed by its product + accumulator, so only ~one quarter of
            # work trails the end of the ACT stream
            tl = T - 1
            ut = uts[tl]
            Q = (2 * C) // NQ
            sgt = sgp.tile([P, 2 * C], fp16, tag="sg")
            for q in range(NQ):
                qs = slice(q * Q, (q + 1) * Q)
                us = slice((q * Q + C) % (2 * C), (q * Q + C) % (2 * C) + Q)
                nc.scalar.activation(sgt[:, qs], ut[:, qs], AF.Exp)
                qm = qp.tile([P, Q], fp16, tag=f"qmq{q}")
                nc.vector.tensor_mul(qm[:], sgt[:, qs], ut[:, us])
                acc(cM + tl + q, qm[:])

            nc.gpsimd.dma_start(outd, outt[:])

    nc.compile()
    return nc


def _get_nc():
    if "nc" not in _cache:
        _cache["nc"] = _build()
    return _cache["nc"]


def kernel(out1, out2, para, target, pos_idx):
    from concourse.bass_utils import run_bass_kernel_spmd

    nc = _get_nc()

    out1 = np.ascontiguousarray(out1, dtype=np.float32)
    out2 = np.ascontiguousarray(out2, dtype=np.float32)
    x1h = out1.astype(np.float16)
    x2h = out2.astype(np.float16)

    in_maps = [
        {
            "x1": x1h[c * RPC : (c + 1) * RPC],
            "x2": x2h[c * RPC : (c + 1) * RPC],
        }
        for c in range(NCORES)
    ]
    res = run_bass_kernel_spmd(nc, in_maps, core_ids=list(range(NCORES)))
    parts = np.stack([r["out"] for r in res.results])  # [NCORES, P, OUTW]

    def rows(col0):
        # device cols [col0 : col0+T], laid out [core, p, t] -> row c*RPC+t*P+p
        return (
            parts[:, :, col0 : col0 + T]
            .transpose(0, 2, 1)
            .reshape(B)
            .astype(np.float64)
        )

    E1, E2 = rows(0), rows(T)
    G12, G21 = rows(2 * T), rows(3 * T)
    Msum = parts[:, :, 4 * T :].sum(dtype=np.float64)

    # host assembly over [B, K] in fp64 (g already gathered on host anyway)
    idx = pos_idx.astype(np.int64)
    g1 = np.take_along_axis(out1, idx, axis=1).astype(np.float64)
    g2 = np.take_along_axis(out2, idx, axis=1).astype(np.float64)
    a1, a2 = np.exp(g1), np.exp(g2)
    D1 = (E1 - a1.sum(1))[:, None] + a1
    D2 = (E2 - a2.sum(1))[:, None] + a2
    r1, r2 = 1.0 / D1, 1.0 / D2
    row_single = (
        np.log(D1).sum(1) + np.log(D2).sum(1)
        - (G12 - (a1 * g2).sum(1)) * r1.sum(1)
        - (G21 - (a2 * g1).sum(1)) * r2.sum(1)
        - (a1 * g2 * r1).sum(1)
        - (a2 * g1 * r2).sum(1)
    )
    single = row_single.sum() / (B * K)
    multi = -Msum / B
    p = float(np.asarray(para))
    return np.asarray(p * multi + (1.0 - p) * single, dtype=np.float32)


# revision 6
# speedup vs baseline: 1.4943x; 1.0621x over previous
"""Trainium2 Bass kernel for nn_DUDCLoss_1382979469646.

Data-parallel over the batch dim: 8 cores x 512 rows each. The loss is
factorized so each row needs only a handful of C-length passes; the
device computes five per-row reductions and the host finishes the tiny
O(B*K) assembly in fp64 (mirroring the host-side gather the input
already requires).

Single (masked-softmax) part: with A=exp(x), E=sum(A), a_k=A[pos_k],
En=E-sum_k(a_k), D_j=En+a_j, the per-(row,j) cross-entropy is
  xent12_j = ln(D2_j) - (G12 - S12 + a1_j*g2_j) / D1_j
where G12 = sum_c A1_c * x2_c and S12 = sum_k a1_k * g2_k. This uses
ln(p+eps) ~= ln(p) (drops the +eps inside the log); measured total error
~1.3e-3 relative - well inside the 2e-2 gate. Dropping eps makes the
log of the softmax numerator the input logit itself, so no ln pass is
needed for the cross terms. Device ships E1,E2,G12,G21 per row; host
(which already gathered g = x[pos]) forms D, ln(D), 1/D over [B,K].

Multi (sigmoid) part: ln(sigmoid(x)+eps) ~= u = x - ln(1+A), and
sigmoid(x) = exp(u) computed as pow(e, u) on the gpsimd engine - so the
ACT engine runs only two passes (Exp, Ln[1+A]), both in the one ACT
table set that holds them (a patched table-selection policy guarantees
a single ~1.3us table load). Device ships sum(s1*u2 + s2*u1) per tile.

Engine split per core (cost model): ACT 2 passes ~17us; Pool (gpsimd)
u-subtract + pow ~14us; DVE row accumulators (tensor_scalar accum, 4x
perf mode) + G/M products (tensor_tensor, 2x mode) ~17us; DMA ~6us
(fp16 inputs, converted on host - fp16 also keeps DVE operands 2-byte
for the fast modes). The last tile's pow/products are split in halves
so only ~1.5us of work trails the ACT/Pool streams.
"""

import numpy as np

NCORES = 8
B, C, K = 4096, 1024, 8
RPC = B // NCORES          # rows per core
P = 128                    # partitions
T = RPC // P               # row-tiles per core
TK = T * K
EPS = 1e-5
OUTW = 4 * T + (T - 1) + 2  # E1,E2,G12,G21 | M per tile | M half-splits

_cache = {}


def _patch_act_tables(mybir, bacc):
    """Make the ACT-table-load inserter resolve both Exp and Ln to the one
    set that holds both (natural_log_exp_and_others). The default policy
    picks a singleton set per function, inserting a ~1.3us table load at
    every Exp<->Ln transition in the scheduled stream."""
    if getattr(bacc, "_dudc_act_patch", False):
        return
    orig = bacc.get_activation_tables
    both = {mybir.ActivationFunctionType.Exp, mybir.ActivationFunctionType.Ln}

    def patched(arch):
        tabs = orig(arch)
        if any(both <= funcs for funcs in tabs.values()):
            for name, funcs in tabs.items():
                if not both <= funcs:
                    funcs.difference_update(both)
        return tabs

    bacc.get_activation_tables = patched
    bacc._dudc_act_patch = True


def _build():
    import concourse.bass as bass
    import concourse.tile as tile
    from concourse import bacc, mybir

    _patch_act_tables(mybir, bacc)

    fp32 = mybir.dt.float32
    fp16 = mybir.dt.float16
    AF = mybir.ActivationFunctionType
    ALU = mybir.AluOpType

    nc = bacc.Bacc(
        "TRN2",
        target_bir_lowering=False,
        debug=False,
        num_devices=NCORES,
    )

    x1d = nc.dram_tensor("x1", [RPC, C], fp16, kind="ExternalInput").ap()
    x2d = nc.dram_tensor("x2", [RPC, C], fp16, kind="ExternalInput").ap()
    outd = nc.dram_tensor("out", [P, OUTW], fp32, kind="ExternalOutput").ap()

    cE1, cE2, cG12, cG21, cM = 0, T, 2 * T, 3 * T, 4 * T

    with tile.TileContext(nc) as tc:
        with (
            tc.tile_pool(name="x", bufs=3) as xp,
            tc.tile_pool(name="A", bufs=2) as ap_,
            tc.tile_pool(name="llp", bufs=2) as llpp,
            tc.tile_pool(name="u", bufs=2) as up,
            tc.tile_pool(name="sg", bufs=2) as sgp,
            tc.tile_pool(name="q", bufs=3) as qp,
            tc.tile_pool(name="scratch", bufs=4) as scp,
            tc.tile_pool(name="small", bufs=1) as sm,
        ):
            outt = sm.tile([P, OUTW], fp32)

            # base-e constant for pow(e, u) = exp(u) on gpsimd; memset runs
            # on the otherwise-idle DVE during the first input DMA
            et = sm.tile([P, 2 * C], fp16)
            nc.vector.memset(et[:], float(np.e))

            # primer: a no-dependency ACT instruction so the ~1.3us ACT table
            # load (inserted before the first activation in the scheduled
            # stream) runs at t=0 instead of behind the first input DMA
            dm = sm.tile([P, 1], fp32)
            dmo = sm.tile([P, 1], fp32)
            nc.vector.memset(dm[:], 0.0)
            nc.scalar.activation(dmo[:], dm[:], AF.Exp)

            def acc(dst_col, src_ap):
                # fast row accumulate: tensor_scalar keeps 4x perf mode
                scw = scp.tile([P, 2 * C], fp16, tag="scw")
                w = src_ap.shape[-1]
                nc.vector.tensor_scalar(
                    scw[:, 0:w], src_ap, 1.0, 0.0, op0=ALU.mult, op1=ALU.add,
                    accum_out=outt[:, dst_col : dst_col + 1],
                )

            uts = []
            for t in range(T):
                r0, r1 = t * P, (t + 1) * P
                if t == 0:
                    # tile 0 on two separate tiles: per-tensor deps so exp of
                    # the x2 half starts as soon as its own DMA lands
                    xta = xp.tile([P, C], fp16, tag="xa")
                    xtb = xp.tile([P, C], fp16, tag="xb")
                    nc.sync.dma_start(xtb[:], x2d[r0:r1, :])
                    nc.sync.dma_start(xta[:], x1d[r0:r1, :])
                    xparts = [(xtb, slice(C, 2 * C)), (xta, slice(0, C))]
                    x1v, x2v = xta[:], xtb[:]
                else:
                    xt = xp.tile([P, 2 * C], fp16, tag="x")
                    nc.sync.dma_start(xt[:, 0:C], x1d[r0:r1, :])
                    nc.sync.dma_start(xt[:, C : 2 * C], x2d[r0:r1, :])
                    xparts = [(xt, slice(0, 2 * C))]
                    x1v, x2v = xt[:, 0:C], xt[:, C : 2 * C]

                At = ap_.tile([P, 2 * C], fp16, tag="A")
                for xsrc, dsl in xparts:
                    nc.scalar.activation(At[:, dsl], xsrc[:], AF.Exp)

                # row sums E1, E2 straight into out columns
                acc(cE1 + t, At[:, 0:C])
                acc(cE2 + t, At[:, C : 2 * C])

                # cross products G12 = sum A1*x2, G21 = sum A2*x1
                qg = qp.tile([P, 2 * C], fp16, tag="qg")
                nc.vector.tensor_mul(qg[:, 0:C], At[:, 0:C], x2v)
                nc.vector.tensor_mul(qg[:, C : 2 * C], At[:, C : 2 * C], x1v)
                acc(cG12 + t, qg[:, 0:C])
                acc(cG21 + t, qg[:, C : 2 * C])

                # u = x - ln(1+A) = log(sigmoid(x)); subtract and
                # s = exp(u) = pow(e, u) both on the gpsimd engine
                LLpt = llpp.tile([P, 2 * C], fp32, tag="llp")
                nc.scalar.activation(LLpt[:], At[:], AF.Ln, bias=1.0)
                ut = up.tile([P, 2 * C], fp16, tag="u")
                sgt = sgp.tile([P, 2 * C], fp16, tag="sg")
                halves = t == 0 or t == T - 1
                if t == 0:
                    for xsrc, dsl in xparts:
                        nc.gpsimd.tensor_sub(ut[:, dsl], xsrc[:], LLpt[:, dsl])
                elif halves:
                    for h in (slice(0, C), slice(C, 2 * C)):
                        nc.gpsimd.tensor_sub(ut[:, h], xt[:, h], LLpt[:, h])
                else:
                    nc.gpsimd.tensor_sub(ut[:], xt[:], LLpt[:])
                if halves:
                    for h in (slice(0, C), slice(C, 2 * C)):
                        nc.gpsimd.tensor_tensor(
                            sgt[:, h], et[:, h], ut[:, h], op=ALU.pow
                        )
                else:
                    nc.gpsimd.tensor_tensor(sgt[:], et[:], ut[:], op=ALU.pow)
                uts.append((ut, sgt))

                # multi products: qm = s * u_swapped; the M12+M21 total goes
                # into one accumulator per tile (only the total enters loss);
                # last tile per half into separate columns to shrink the tail
                if t < T - 1:
                    qm = qp.tile([P, 2 * C], fp16, tag="qm")
                    nc.vector.tensor_mul(qm[:, 0:C], sgt[:, 0:C], ut[:, C : 2 * C])
                    nc.vector.tensor_mul(
                        qm[:, C : 2 * C], sgt[:, C : 2 * C], ut[:, 0:C]
                    )
                    acc(cM + t, qm[:])
                else:
                    qma = qp.tile([P, C], fp16, tag="qma")
                    nc.vector.tensor_mul(qma[:], sgt[:, 0:C], ut[:, C : 2 * C])
                    acc(cM + t, qma[:])
                    qmb = qp.tile([P, C], fp16, tag="qmb")
                    nc.vector.tensor_mul(qmb[:], sgt[:, C : 2 * C], ut[:, 0:C])
                    acc(cM + t + 1, qmb[:])

            nc.gpsimd.dma_start(outd, outt[:])

    nc.compile()
    return nc


def _get_nc():
    if "nc" not in _cache:
        _cache["nc"] = _build()
    return _cache["nc"]


def kernel(out1, out2, para, target, pos_idx):
    from concourse.bass_utils import run_bass_kernel_spmd

    nc = _get_nc()

    out1 = np.ascontiguousarray(out1, dtype=np.float32)
    out2 = np.ascontiguousarray(out2, dtype=np.float32)
    x1h = out1.astype(np.float16)
    x2h = out2.astype(np.float16)

    in_maps = [
        {
            "x1": x1h[c * RPC : (c + 1) * RPC],
            "x2": x2h[c * RPC : (c + 1) * RPC],
        }
        for c in range(NCORES)
    ]
    res = run_bass_kernel_spmd(nc, in_maps, core_ids=list(range(NCORES)))
    parts = np.stack([r["out"] for r in res.results])  # [NCORES, P, OUTW]

    def rows(col0):
        # device cols [col0 : col0+T], laid out [core, p, t] -> row c*RPC+t*P+p
        return (
            parts[:, :, col0 : col0 + T]
            .transpose(0, 2, 1)
            .reshape(B)
            .astype(np.float64)
        )

    E1, E2 = rows(0), rows(T)
    G12, G21 = rows(2 * T), rows(3 * T)
    Msum = parts[:, :, 4 * T :].sum(dtype=np.float64)

    # host assembly over [B, K] in fp64 (g already gathered on host anyway)
    idx = pos_idx.astype(np.int64)
    g1 = np.take_along_axis(out1, idx, axis=1).astype(np.float64)
    g2 = np.take_along_axis(out2, idx, axis=1).astype(np.float64)
    a1, a2 = np.exp(g1), np.exp(g2)
    D1 = (E1 - a1.sum(1))[:, None] + a1
    D2 = (E2 - a2.sum(1))[:, None] + a2
    r1, r2 = 1.0 / D1, 1.0 / D2
    row_single = (
        np.log(D1).sum(1) + np.log(D2).sum(1)
        - (G12 - (a1 * g2).sum(1)) * r1.sum(1)
        - (G21 - (a2 * g1).sum(1)) * r2.sum(1)
        - (a1 * g2 * r1).sum(1)
        - (a2 * g1 * r2).sum(1)
    )
    single = row_single.sum() / (B * K)
    multi = -Msum / B
    p = float(np.asarray(para))
    return np.asarray(p * multi + (1.0 - p) * single, dtype=np.float32)


# revision 7
# speedup vs baseline: 1.5662x; 1.0481x over previous
"""Trainium2 Bass kernel for nn_DUDCLoss_1382979469646.

Data-parallel over the batch dim: 8 cores x 512 rows each. The loss is
factorized so each row needs only a handful of C-length passes; the
device computes five per-row reductions and the host finishes the tiny
O(B*K) assembly in fp64 (mirroring the host-side gather the input
already requires).

Single (masked-softmax) part: with A=exp(x), E=sum(A), a_k=A[pos_k],
En=E-sum_k(a_k), D_j=En+a_j, the per-(row,j) cross-entropy is
  xent12_j = ln(D2_j) - (G12 - S12 + a1_j*g2_j) / D1_j
where G12 = sum_c A1_c * x2_c and S12 = sum_k a1_k * g2_k. This uses
ln(p+eps) ~= ln(p) (drops the +eps inside the log); measured total error
~1.2e-3 relative - well inside the 2e-2 gate. Dropping eps makes the
log of the softmax numerator the input logit itself, so no ln pass is
needed for the cross terms. Device ships E1,E2,G12,G21 per row; host
(which already gathered g = x[pos]) forms D, ln(D), 1/D over [B,K].

Multi (sigmoid) part: ln(sigmoid(x)+eps) ~= u = x - ln(1+A), and
sigmoid(x) = exp(u) computed as pow(e, u) on the gpsimd engine - so the
ACT engine runs only two big passes (Exp, Ln[1+A]), both in the one ACT
table set that holds them (a patched table-selection policy guarantees
a single ~1.3us table load). Device ships sum(s1*u2 + s2*u1) per tile.

Scheduling: the tile scheduler orders each engine's queue by readiness,
which lets exp(t+1) preempt ln1p(t) and starves the downstream
gpsimd/DVE pipeline - so exp(t+1) carries a zero bias AP produced (on
the prompt gpsimd queue) from ln1p(t)'s output, forcing the
exp/ln1p/exp... alternation. The last tile runs in quarter-columns:
u-subtract quarters first, sigmoid quarters split between ACT exp(u)
(idle by then) and gpsimd pow, each followed immediately by its product
and accumulator so only ~1us trails the engine streams.

Engine budget per core (cost model): ACT ~16us, DVE ~18us (row
accumulators via tensor_scalar accum in 4x perf mode, products via
tensor_tensor in 2x - all operands 2-byte), Pool ~15us, DMA ~6us (fp16
inputs, converted on host).
"""

import numpy as np

NCORES = 8
B, C, K = 4096, 1024, 8
RPC = B // NCORES          # rows per core
P = 128                    # partitions
T = RPC // P               # row-tiles per core
TK = T * K
EPS = 1e-5
OUTW = 4 * T + (T - 1) + 4  # E1,E2,G12,G21 | M per tile | M quarter-splits

_cache = {}


def _patch_act_tables(mybir, bacc):
    """Make the ACT-table-load inserter resolve both Exp and Ln to the one
    set that holds both (natural_log_exp_and_others). The default policy
    picks a singleton set per function, inserting a ~1.3us table load at
    every Exp<->Ln transition in the scheduled stream."""
    if getattr(bacc, "_dudc_act_patch", False):
        return
    orig = bacc.get_activation_tables
    both = {mybir.ActivationFunctionType.Exp, mybir.ActivationFunctionType.Ln}

    def patched(arch):
        tabs = orig(arch)
        if any(both <= funcs for funcs in tabs.values()):
            for name, funcs in tabs.items():
                if not both <= funcs:
                    funcs.difference_update(both)
        return tabs

    bacc.get_activation_tables = patched
    bacc._dudc_act_patch = True


def _build():
    import concourse.bass as bass
    import concourse.tile as tile
    from concourse import bacc, mybir

    _patch_act_tables(mybir, bacc)

    fp32 = mybir.dt.float32
    fp16 = mybir.dt.float16
    AF = mybir.ActivationFunctionType
    ALU = mybir.AluOpType

    nc = bacc.Bacc(
        "TRN2",
        target_bir_lowering=False,
        debug=False,
        num_devices=NCORES,
    )

    x1d = nc.dram_tensor("x1", [RPC, C], fp16, kind="ExternalInput").ap()
    x2d = nc.dram_tensor("x2", [RPC, C], fp16, kind="ExternalInput").ap()
    outd = nc.dram_tensor("out", [P, OUTW], fp32, kind="ExternalOutput").ap()

    cE1, cE2, cG12, cG21, cM = 0, T, 2 * T, 3 * T, 4 * T

    with tile.TileContext(nc) as tc:
        with (
            tc.tile_pool(name="x", bufs=4) as xp,
            tc.tile_pool(name="A", bufs=3) as ap_,
            tc.tile_pool(name="llp", bufs=3) as llpp,
            tc.tile_pool(name="u", bufs=3) as up,
            tc.tile_pool(name="sg", bufs=3) as sgp,
            tc.tile_pool(name="q", bufs=4) as qp,
            tc.tile_pool(name="scratch", bufs=6) as scp,
            tc.tile_pool(name="small", bufs=1) as sm,
        ):
            outt = sm.tile([P, OUTW], fp32)

            # base-e constant for pow(e, u) = exp(u) on gpsimd; memset runs
            # on the otherwise-idle DVE during the first input DMA
            et = sm.tile([P, 2 * C], fp16)
            nc.vector.memset(et[:], float(np.e))

            # primer: a no-dependency ACT instruction so the ~1.3us ACT table
            # load (inserted before the first activation in the scheduled
            # stream) runs at t=0 instead of behind the first input DMA
            dm = sm.tile([P, 1], fp32)
            dmo = sm.tile([P, 1], fp32)
            nc.vector.memset(dm[:], 0.0)
            nc.scalar.activation(dmo[:], dm[:], AF.Exp)

            def acc(dst_col, src_ap):
                # fast row accumulate: tensor_scalar keeps 4x perf mode
                scw = scp.tile([P, 2 * C], fp16, tag="scw")
                w = src_ap.shape[-1]
                nc.vector.tensor_scalar(
                    scw[:, 0:w], src_ap, 1.0, 0.0, op0=ALU.mult, op1=ALU.add,
                    accum_out=outt[:, dst_col : dst_col + 1],
                )

            zbs = []   # [P,1] zero bias tiles forcing exp(t+1) after ln1p(t)
            uts = []
            for t in range(T):
                r0, r1 = t * P, (t + 1) * P
                if t == 0:
                    # tile 0 on two separate tiles: per-tensor deps so exp of
                    # the x2 half starts as soon as its own DMA lands
                    xta = xp.tile([P, C], fp16, tag="xa")
                    xtb = xp.tile([P, C], fp16, tag="xb")
                    nc.sync.dma_start(xtb[:], x2d[r0:r1, :])
                    nc.sync.dma_start(xta[:], x1d[r0:r1, :])
                    xparts = [(xtb, slice(C, 2 * C)), (xta, slice(0, C))]
                    x1v, x2v = xta[:], xtb[:]
                else:
                    xt = xp.tile([P, 2 * C], fp16, tag="x")
                    nc.sync.dma_start(xt[:, 0:C], x1d[r0:r1, :])
                    nc.sync.dma_start(xt[:, C : 2 * C], x2d[r0:r1, :])
                    xparts = [(xt, slice(0, 2 * C))]
                    x1v, x2v = xt[:, 0:C], xt[:, C : 2 * C]

                At = ap_.tile([P, 2 * C], fp16, tag="A")
                if t == 0:
                    for xsrc, dsl in xparts:
                        nc.scalar.activation(At[:, dsl], xsrc[:], AF.Exp)
                else:
                    # zero bias from ln1p(t-1) pins queue order exp/ln1p/...
                    nc.scalar.activation(At[:], xt[:], AF.Exp, bias=zbs[t - 1][:])

                # row sums E1, E2 straight into out columns
                acc(cE1 + t, At[:, 0:C])
                acc(cE2 + t, At[:, C : 2 * C])

                # cross products G12 = sum A1*x2, G21 = sum A2*x1; tile 0's
                # multiplies fill the gpsimd queue's initial idle window
                qg = qp.tile([P, 2 * C], fp16, tag="qg")
                eng = nc.gpsimd if t == 0 else nc.vector
                eng.tensor_tensor(qg[:, 0:C], At[:, 0:C], x2v, op=ALU.mult)
                eng.tensor_tensor(qg[:, C : 2 * C], At[:, C : 2 * C], x1v, op=ALU.mult)
                acc(cG12 + t, qg[:, 0:C])
                acc(cG21 + t, qg[:, C : 2 * C])

                # u = x - ln(1+A) = log(sigmoid(x)); subtract and
                # s = exp(u) = pow(e, u) on the gpsimd engine
                LLpt = llpp.tile([P, 2 * C], fp32, tag="llp")
                nc.scalar.activation(LLpt[:], At[:], AF.Ln, bias=1.0)
                if t < T - 1:
                    zb = sm.tile([P, 1], fp32)
                    nc.gpsimd.tensor_scalar(
                        zb[:], LLpt[:, 0:1], 0.0, None, op0=ALU.mult
                    )
                    zbs.append(zb)
                ut = up.tile([P, 2 * C], fp16, tag="u")
                sgt = sgp.tile([P, 2 * C], fp16, tag="sg")
                if t == 0:
                    for xsrc, dsl in xparts:
                        nc.gpsimd.tensor_sub(ut[:, dsl], xsrc[:], LLpt[:, dsl])
                    for h in (slice(0, C), slice(C, 2 * C)):
                        nc.gpsimd.tensor_tensor(
                            sgt[:, h], et[:, h], ut[:, h], op=ALU.pow
                        )
                elif t < T - 1:
                    nc.gpsimd.tensor_sub(ut[:], xt[:], LLpt[:])
                    nc.gpsimd.tensor_tensor(sgt[:], et[:], ut[:], op=ALU.pow)
                uts.append((ut, sgt))

                # multi products: qm = s * u_swapped; M12+M21 into one
                # accumulator per tile (only the total enters the loss)
                if t < T - 1:
                    qm = qp.tile([P, 2 * C], fp16, tag="qm")
                    nc.vector.tensor_mul(qm[:, 0:C], sgt[:, 0:C], ut[:, C : 2 * C])
                    nc.vector.tensor_mul(
                        qm[:, C : 2 * C], sgt[:, C : 2 * C], ut[:, 0:C]
                    )
                    acc(cM + t, qm[:])
                else:
                    # last tile in quarter-columns. u quarters first (q2,q4
                    # feed the ACT exp(u) quarters; q1,q3 the gpsimd pow),
                    # then each sigmoid quarter -> product -> accumulator.
                    Q = (2 * C) // 4
                    qs = [slice(i * Q, (i + 1) * Q) for i in range(4)]
                    for i in (1, 3, 0, 2):
                        nc.gpsimd.tensor_sub(ut[:, qs[i]], xt[:, qs[i]], LLpt[:, qs[i]])
                    # ACT sigmoid quarters (engine idle after last ln1p)
                    for i in (1, 3):
                        nc.scalar.activation(sgt[:, qs[i]], ut[:, qs[i]], AF.Exp)
                    # gpsimd sigmoid quarters
                    for i in (0, 2):
                        nc.gpsimd.tensor_tensor(
                            sgt[:, qs[i]], et[:, qs[i]], ut[:, qs[i]], op=ALU.pow
                        )
                    # products in availability order; partner quarter (i+2)%4
                    for n, i in enumerate((1, 3, 0, 2)):
                        j = (i + 2) % 4
                        qm = qp.tile([P, Q], fp16, tag=f"qmq{i}")
                        nc.vector.tensor_mul(qm[:], sgt[:, qs[i]], ut[:, qs[j]])
                        acc(cM + t + n, qm[:])

            nc.gpsimd.dma_start(outd, outt[:])

    nc.compile()
    return nc


def _get_nc():
    if "nc" not in _cache:
        _cache["nc"] = _build()
    return _cache["nc"]


def kernel(out1, out2, para, target, pos_idx):
    from concourse.bass_utils import run_bass_kernel_spmd

    nc = _get_nc()

    out1 = np.ascontiguousarray(out1, dtype=np.float32)
    out2 = np.ascontiguousarray(out2, dtype=np.float32)
    x1h = out1.astype(np.float16)
    x2h = out2.astype(np.float16)

    in_maps = [
        {
            "x1": x1h[c * RPC : (c + 1) * RPC],
            "x2": x2h[c * RPC : (c + 1) * RPC],
        }
        for c in range(NCORES)
    ]
    res = run_bass_kernel_spmd(nc, in_maps, core_ids=list(range(NCORES)))
    parts = np.stack([r["out"] for r in res.results])  # [NCORES, P, OUTW]

    def rows(col0):
        # device cols [col0 : col0+T], laid out [core, p, t] -> row c*RPC+t*P+p
        return (
            parts[:, :, col0 : col0 + T]
            .transpose(0, 2, 1)
            .reshape(B)
            .astype(np.float64)
        )

    E1, E2 = rows(0), rows(T)
    G12, G21 = rows(2 * T), rows(3 * T)
    Msum = parts[:, :, 4 * T :].sum(dtype=np.float64)

    # host assembly over [B, K] in fp64 (g already gathered on host anyway)
    idx = pos_idx.astype(np.int64)
    g1 = np.take_along_axis(out1, idx, axis=1).astype(np.float64)
    g2 = np.take_along_axis(out2, idx, axis=1).astype(np.float64)
    a1, a2 = np.exp(g1), np.exp(g2)
    D1 = (E1 - a1.sum(1))[:, None] + a1
    D2 = (E2 - a2.sum(1))[:, None] + a2
    r1, r2 = 1.0 / D1, 1.0 / D2
    row_single = (
        np.log(D1).sum(1) + np.log(D2).sum(1)
        - (G12 - (a1 * g2).sum(1)) * r1.sum(1)
        - (G21 - (a2 * g1).sum(1)) * r2.sum(1)
        - (a1 * g2 * r1).sum(1)
        - (a2 * g1 * r2).sum(1)
    )
    single = row_single.sum() / (B * K)
    multi = -Msum / B
    p = float(np.asarray(para))
    return np.asarray(p * multi + (1.0 - p) * single, dtype=np.float32)
